# revision 4
# baseline (speedup 1.0000x reference)
"""Trainium2 Bass kernel for nn_CA_Model (neural cellular automaton).

Strategy: pure data-parallel over batch (8 images -> 8 cores). Per core, the
whole [256,256,16] image lives in SBUF in an interleaved layout
  Xc[p, f]: p = (row%8)*16 + ch, f = (row//8)*258 + 1 + w
with zero padding columns (w=-1,256) and zero pad rows (row 0, 257; image rows
are 1..256). The 3x3 depthwise perceive + first MLP layer fuse into three
PSUM-accumulated matmuls per output row (one per horizontal tap dj): the
vertical taps come from the lhsT's placement of per-tap weight blocks
A[di,dj] on the partition rows of the enclosing 8-row block. All matmul
APs are full-128-partition (base 0), which satisfies the TRN2 quadrant
constraints; row selection happens through zero rows in lhsT.

Layer 2 accumulates eight per-row matmuls (zero-padded W1^T columns) into one
PSUM tile that lands directly in the Xc layout. Alive-mask maxpools run on a
rows-on-partitions alpha tile: horizontal max via shifted free-dim APs,
vertical max via DMA partition-shifted copies.
"""
import sys
for _p in ("/opt/trn_rl_repo", "/root/.axon_site/_ro/trn_rl_repo"):
    if _p not in sys.path:
        sys.path.append(_p)

import numpy as np

C = 16
HID = 128
H = W = 256
NB = 33            # row blocks in Xc layout (33*8 = 264 row slots, rows 0..257 used)
FW = 258           # padded row width in free dim
FSZ = NB * FW      # 8514 free elements per partition
NROW = 256         # image rows (stored at row index 1..256)


def _sobel():
    dx = np.outer([1, 2, 1], [-1, 0, 1]) / 8.0
    f1 = dx.T.astype(np.float32)   # angle=0: F1 = dx.T
    f2 = dx.astype(np.float32)     # F2 = dx
    return f1, f2


def build_weights(W0, b0, W1):
    """Host-side preprocessing of the MLP weights into lhsT tensors."""
    F1, F2 = _sobel()
    W0x, W0y1, W0y2 = W0[:, 0:16], W0[:, 16:32], W0[:, 32:48]
    # A[di][dj]: [HID, C] applied to x[row-1+di, w-1+dj]
    A = [[(np.float32(di == 1 and dj == 1) * W0x
           + F1[di, dj] * W0y1 + F2[di, dj] * W0y2).astype(np.float32)
          for dj in range(3)] for di in range(3)]

    # layer-1 lhsT variants. Key: (s, dj, part) where part is 0 for the lhsT
    # covering the block holding row rho-1 (and possibly rho, rho+1), 1 for
    # the spill into block t+1 (s in {6,7}).
    l1 = {}
    for dj in range(3):
        for s in range(6):
            L = np.zeros((128, 128), np.float32)
            for di in range(3):
                # lhsT[16*(s+di)+c, m] = A[di][dj][m, c]
                L[16 * (s + di):16 * (s + di) + 16, :] = A[di][dj].T
            l1[(s, dj, 0)] = L
        # s == 6: rows rho-1 (g6), rho (g7) in block t; rho+1 (g0) in t+1
        L = np.zeros((128, 128), np.float32)
        L[96:112, :] = A[0][dj].T
        L[112:128, :] = A[1][dj].T
        l1[(6, dj, 0)] = L
        L = np.zeros((128, 128), np.float32)
        L[0:16, :] = A[2][dj].T
        l1[(6, dj, 1)] = L
        # s == 7: row rho-1 (g7) in block t; rho (g0), rho+1 (g1) in t+1
        L = np.zeros((128, 128), np.float32)
        L[112:128, :] = A[0][dj].T
        l1[(7, dj, 0)] = L
        L = np.zeros((128, 128), np.float32)
        L[0:16, :] = A[1][dj].T
        L[16:32, :] = A[2][dj].T
        l1[(7, dj, 1)] = L

    # layer-2 lhsT: for a row with group g, W1pad[g][n, 16g+c] = W1[c, n]
    w1p = []
    for g in range(8):
        Wp = np.zeros((128, 128), np.float32)
        Wp[:, 16 * g:16 * g + 16] = W1.T
        w1p.append(Wp)

    # life-broadcast lhsT variants: for block tb, rows rho=8tb+g (real rows
    # only), life value lives at LifeQ[q = rho-1, half = q//128].
    # Group rows into buckets by (half, 64-aligned window base). lhsT
    # R[b + (qh-b), 16g+c] = 1 -> out[16g+c, w] = life[row 8tb+g].
    life_plan = []   # per tb: list of (half, lhsT_index); lhsT K=128 base 0
    r_mats = []
    for tb in range(NB):
        plan = []
        buckets = {}
        for g in range(8):
            rho = 8 * tb + g
            if rho < 1 or rho > 256:
                continue
            q = rho - 1
            half, qh = q // 128, q % 128
            buckets.setdefault(half, []).append((g, qh))
        for half, rows in sorted(buckets.items()):
            Rm = np.zeros((128, 128), np.float32)
            for g, qh in rows:
                Rm[qh, 16 * g:16 * g + 16] = 1.0
            plan.append((half, len(r_mats)))
            r_mats.append(Rm)
        life_plan.append(plan)

    l1_keys = sorted(l1.keys())
    l1_idx = {k: i for i, k in enumerate(l1_keys)}
    l1_stack = np.stack([l1[k] for k in l1_keys])          # [NL1, 128, 128]
    w1_stack = np.stack(w1p)                               # [8, 128, 128]
    r_stack = np.stack(r_mats)                             # [NR, 128, 128]
    return dict(l1_stack=l1_stack, l1_idx=l1_idx, w1_stack=w1_stack,
                r_stack=r_stack, life_plan=life_plan,
                b0=b0.reshape(128, 1).astype(np.float32))


def marshal_x(img):
    """[256,256,16] image -> Xc [128, FSZ] interleaved layout."""
    xp = np.zeros((NB * 8, FW, C), np.float32)
    xp[1:257, 1:257, :] = img
    # Xc[g*16+c, t*258+w] = xp[8t+g, w, c]
    xc = xp.reshape(NB, 8, FW, C).transpose(1, 3, 0, 2).reshape(128, FSZ)
    return np.ascontiguousarray(xc)


def unmarshal_x(xc):
    """Xc [128, FSZ] -> [256,256,16] image."""
    xp = xc.reshape(8, C, NB, FW).transpose(2, 0, 3, 1)   # [NB, 8, FW, C]
    xp = xp.reshape(NB * 8, FW, C)
    return np.ascontiguousarray(xp[1:257, 1:257, :])


_PROGRAM_CACHE = {}


def kernel(x, W0, b0, W1, steps, _trace=False):
    import concourse.bass_utils as bass_utils
    steps = int(steps)
    x = np.asarray(x, dtype=np.float32)
    W0 = np.asarray(W0, dtype=np.float32)
    b0 = np.asarray(b0, dtype=np.float32)
    W1 = np.asarray(W1, dtype=np.float32)
    B = x.shape[0]
    assert x.shape == (8, H, W, C), x.shape

    wts = build_weights(W0, b0, W1)
    key = steps
    if key not in _PROGRAM_CACHE:
        from kernel_program import build_program
        _PROGRAM_CACHE[key] = build_program(steps, wts["l1_idx"],
                                            wts["life_plan"],
                                            wts["l1_stack"].shape[0],
                                            wts["r_stack"].shape[0])
    nc = _PROGRAM_CACHE[key]

    in_maps = []
    for b in range(B):
        in_maps.append({
            "xc": marshal_x(x[b]),
            "l1w": wts["l1_stack"],
            "w1w": wts["w1_stack"],
            "rw": wts["r_stack"],
            "b0w": wts["b0"],
        })
    res = bass_utils.run_bass_kernel_spmd(nc, in_maps, list(range(8)),
                                          trace=_trace)
    kernel.last_result = res
    out = np.stack([unmarshal_x(res.results[b]["out"]) for b in range(B)])
    return out.astype(np.float32)


# Allow `from kernel import *` helpers for the program builder living in a
# separate module during development; the final self-contained kernel.py
# inlines build_program (see kernel_program.py merge step).
if __name__ == "__main__":
    pass


# revision 5
# speedup vs baseline: 1.1271x; 1.1271x over previous
"""Trainium2 Bass kernel for nn_CA_Model (neural cellular automaton).

Strategy: pure data-parallel over batch (8 images -> 8 cores). Per core, the
whole [256,256,16] image lives in SBUF in an interleaved layout
  Xc[p, f]: p = (row%8)*16 + ch, f = (row//8)*258 + 1 + w
with zero padding columns (w=-1,256) and zero pad rows (row 0, 257; image rows
are 1..256). The 3x3 depthwise perceive + first MLP layer fuse into three
PSUM-accumulated matmuls per output row (one per horizontal tap dj): the
vertical taps come from the lhsT's placement of per-tap weight blocks
A[di,dj] on the partition rows of the enclosing 8-row block. All matmul
APs are full-128-partition (base 0), which satisfies the TRN2 quadrant
constraints; row selection happens through zero rows in lhsT.

Layer 2 accumulates eight per-row matmuls (zero-padded W1^T columns) into one
PSUM tile that lands directly in the Xc layout. Alive-mask maxpools run on a
rows-on-partitions alpha tile: horizontal max via shifted free-dim APs,
vertical max via DMA partition-shifted copies.
"""
import sys
for _p in ("/opt/trn_rl_repo", "/root/.axon_site/_ro/trn_rl_repo"):
    if _p not in sys.path:
        sys.path.append(_p)

import numpy as np

C = 16
HID = 128
H = W = 256
NB = 33            # row blocks in Xc layout (33*8 = 264 row slots, rows 0..257 used)
FW = 258           # padded row width in free dim
FSZ = NB * FW      # 8514 free elements per partition
NROW = 256         # image rows (stored at row index 1..256)


def _sobel():
    dx = np.outer([1, 2, 1], [-1, 0, 1]) / 8.0
    f1 = dx.T.astype(np.float32)   # angle=0: F1 = dx.T
    f2 = dx.astype(np.float32)     # F2 = dx
    return f1, f2


def build_weights(W0, b0, W1):
    """Host-side preprocessing of the MLP weights into lhsT tensors."""
    F1, F2 = _sobel()
    W0x, W0y1, W0y2 = W0[:, 0:16], W0[:, 16:32], W0[:, 32:48]
    # A[di][dj]: [HID, C] applied to x[row-1+di, w-1+dj]
    A = [[(np.float32(di == 1 and dj == 1) * W0x
           + F1[di, dj] * W0y1 + F2[di, dj] * W0y2).astype(np.float32)
          for dj in range(3)] for di in range(3)]

    # layer-1 lhsT variants. Key: (s, dj, part) where part is 0 for the lhsT
    # covering the block holding row rho-1 (and possibly rho, rho+1), 1 for
    # the spill into block t+1 (s in {6,7}).
    l1 = {}
    for dj in range(3):
        for s in range(6):
            L = np.zeros((128, 128), np.float32)
            for di in range(3):
                # lhsT[16*(s+di)+c, m] = A[di][dj][m, c]
                L[16 * (s + di):16 * (s + di) + 16, :] = A[di][dj].T
            l1[(s, dj, 0)] = L
        # s == 6: rows rho-1 (g6), rho (g7) in block t; rho+1 (g0) in t+1
        L = np.zeros((128, 128), np.float32)
        L[96:112, :] = A[0][dj].T
        L[112:128, :] = A[1][dj].T
        l1[(6, dj, 0)] = L
        L = np.zeros((128, 128), np.float32)
        L[0:16, :] = A[2][dj].T
        l1[(6, dj, 1)] = L
        # s == 7: row rho-1 (g7) in block t; rho (g0), rho+1 (g1) in t+1
        L = np.zeros((128, 128), np.float32)
        L[112:128, :] = A[0][dj].T
        l1[(7, dj, 0)] = L
        L = np.zeros((128, 128), np.float32)
        L[0:16, :] = A[1][dj].T
        L[16:32, :] = A[2][dj].T
        l1[(7, dj, 1)] = L

    # layer-2 lhsT: for a row with group g, W1pad[g][n, 16g+c] = W1[c, n]
    w1p = []
    for g in range(8):
        Wp = np.zeros((128, 128), np.float32)
        Wp[:, 16 * g:16 * g + 16] = W1.T
        w1p.append(Wp)

    # life-broadcast lhsT variants: for block tb, rows rho=8tb+g (real rows
    # only), life value lives at LifeQ[q = rho-1, half = q//128].
    # Group rows into buckets by (half, 64-aligned window base). lhsT
    # R[b + (qh-b), 16g+c] = 1 -> out[16g+c, w] = life[row 8tb+g].
    life_plan = []   # per tb: list of (half, lhsT_index); lhsT K=128 base 0
    r_mats = []
    for tb in range(NB):
        plan = []
        buckets = {}
        for g in range(8):
            rho = 8 * tb + g
            if rho < 1 or rho > 256:
                continue
            q = rho - 1
            half, qh = q // 128, q % 128
            buckets.setdefault(half, []).append((g, qh))
        for half, rows in sorted(buckets.items()):
            # lhsT indexed by the pool tile's q2 layout: row (q%8)*16 + (q//8)%16
            Rm = np.zeros((128, 128), np.float32)
            for g, qh in rows:
                q2 = (qh % 8) * 16 + qh // 8
                Rm[q2, 16 * g:16 * g + 16] = 1.0
            plan.append((half, len(r_mats)))
            r_mats.append(Rm)
        life_plan.append(plan)

    import ml_dtypes
    bf16 = ml_dtypes.bfloat16
    l1_keys = sorted(l1.keys())
    l1_idx = {k: i for i, k in enumerate(l1_keys)}
    l1_stack = np.stack([l1[k] for k in l1_keys])          # [NL1, 128, 128] f32
    w1_stack = np.stack(w1p).astype(bf16)                  # [8, 128, 128] bf16
    r_stack = np.stack(r_mats).astype(bf16)                # [NR, 128, 128] bf16
    return dict(l1_stack=l1_stack, l1_idx=l1_idx, w1_stack=w1_stack,
                r_stack=r_stack, life_plan=life_plan,
                b0=b0.reshape(128, 1).astype(np.float32))


def marshal_x(img):
    """[256,256,16] image -> Xc [128, FSZ] interleaved layout."""
    xp = np.zeros((NB * 8, FW, C), np.float32)
    xp[1:257, 1:257, :] = img
    # Xc[g*16+c, t*258+w] = xp[8t+g, w, c]
    xc = xp.reshape(NB, 8, FW, C).transpose(1, 3, 0, 2).reshape(128, FSZ)
    return np.ascontiguousarray(xc)


def unmarshal_x(xc):
    """Xc [128, FSZ] -> [256,256,16] image."""
    xp = xc.reshape(8, C, NB, FW).transpose(2, 0, 3, 1)   # [NB, 8, FW, C]
    xp = xp.reshape(NB * 8, FW, C)
    return np.ascontiguousarray(xp[1:257, 1:257, :])


_PROGRAM_CACHE = {}


def kernel(x, W0, b0, W1, steps, _trace=False):
    import concourse.bass_utils as bass_utils
    steps = int(steps)
    x = np.asarray(x, dtype=np.float32)
    W0 = np.asarray(W0, dtype=np.float32)
    b0 = np.asarray(b0, dtype=np.float32)
    W1 = np.asarray(W1, dtype=np.float32)
    B = x.shape[0]
    assert x.shape == (8, H, W, C), x.shape

    wts = build_weights(W0, b0, W1)
    key = steps
    if key not in _PROGRAM_CACHE:
        from kernel_program import build_program
        _PROGRAM_CACHE[key] = build_program(steps, wts["l1_idx"],
                                            wts["life_plan"],
                                            wts["l1_stack"].shape[0],
                                            wts["r_stack"].shape[0])
    nc = _PROGRAM_CACHE[key]

    in_maps = []
    for b in range(B):
        in_maps.append({
            "xc": marshal_x(x[b]),
            "l1w": wts["l1_stack"],
            "w1w": wts["w1_stack"],
            "rw": wts["r_stack"],
            "b0w": wts["b0"],
        })
    res = bass_utils.run_bass_kernel_spmd(nc, in_maps, list(range(8)),
                                          trace=_trace)
    kernel.last_result = res
    out = np.stack([unmarshal_x(res.results[b]["out"]) for b in range(B)])
    return out.astype(np.float32)


# Allow `from kernel import *` helpers for the program builder living in a
# separate module during development; the final self-contained kernel.py
# inlines build_program (see kernel_program.py merge step).
if __name__ == "__main__":
    pass


# revision 6
# speedup vs baseline: 1.2170x; 1.0798x over previous
"""Trainium2 Bass kernel for nn_CA_Model (neural cellular automaton).

Strategy: pure data-parallel over batch (8 images -> 8 cores). Per core, the
whole [256,256,16] image lives in SBUF in an interleaved layout
  Xc[p, f]: p = (row%8)*16 + ch, f = (row//8)*258 + 1 + w
with zero padding columns (w=-1,256) and zero pad rows (row 0, 257; image rows
are 1..256). The 3x3 depthwise perceive + first MLP layer fuse into three
PSUM-accumulated matmuls per output row (one per horizontal tap dj): the
vertical taps come from the lhsT's placement of per-tap weight blocks
A[di,dj] on the partition rows of the enclosing 8-row block. All matmul
APs are full-128-partition (base 0), which satisfies the TRN2 quadrant
constraints; row selection happens through zero rows in lhsT.

Layer 2 accumulates eight per-row matmuls (zero-padded W1^T columns) into one
PSUM tile that lands directly in the Xc layout. Alive-mask maxpools run on a
rows-on-partitions alpha tile: horizontal max via shifted free-dim APs,
vertical max via DMA partition-shifted copies.
"""
import sys
for _p in ("/opt/trn_rl_repo", "/root/.axon_site/_ro/trn_rl_repo"):
    if _p not in sys.path:
        sys.path.append(_p)

import numpy as np

C = 16
HID = 128
H = W = 256
NB = 33            # row blocks in Xc layout (33*8 = 264 row slots, rows 0..257 used)
FW = 258           # padded row width in free dim
FSZ = NB * FW      # 8514 free elements per partition
NROW = 256         # image rows (stored at row index 1..256)


def _sobel():
    dx = np.outer([1, 2, 1], [-1, 0, 1]) / 8.0
    f1 = dx.T.astype(np.float32)   # angle=0: F1 = dx.T
    f2 = dx.astype(np.float32)     # F2 = dx
    return f1, f2


def build_weights(W0, b0, W1):
    """Host-side preprocessing of the MLP weights into lhsT tensors."""
    F1, F2 = _sobel()
    W0x, W0y1, W0y2 = W0[:, 0:16], W0[:, 16:32], W0[:, 32:48]
    # A[di][dj]: [HID, C] applied to x[row-1+di, w-1+dj]
    A = [[(np.float32(di == 1 and dj == 1) * W0x
           + F1[di, dj] * W0y1 + F2[di, dj] * W0y2).astype(np.float32)
          for dj in range(3)] for di in range(3)]

    # layer-1 lhsT variants. Key: (s, dj, part) where part is 0 for the lhsT
    # covering the block holding row rho-1 (and possibly rho, rho+1), 1 for
    # the spill into block t+1 (s in {6,7}).
    l1 = {}
    for dj in range(3):
        for s in range(6):
            L = np.zeros((128, 128), np.float32)
            for di in range(3):
                # lhsT[16*(s+di)+c, m] = A[di][dj][m, c]
                L[16 * (s + di):16 * (s + di) + 16, :] = A[di][dj].T
            l1[(s, dj, 0)] = L
        # s in {6, 7}: single matmul against the X2 (4-row-shifted blocking)
        # copy. Row r sits at X2 partition group (r+4)%8: s=6 -> rows at
        # groups 2,3,4; s=7 -> groups 3,4,5.
        for s, g0 in ((6, 2), (7, 3)):
            L = np.zeros((128, 128), np.float32)
            for di in range(3):
                L[16 * (g0 + di):16 * (g0 + di) + 16, :] = A[di][dj].T
            l1[(s, dj, 0)] = L

    # layer-2 lhsT: for a row with group g, W1pad[g][n, 16g+c] = W1[c, n]
    w1p = []
    for g in range(8):
        Wp = np.zeros((128, 128), np.float32)
        Wp[:, 16 * g:16 * g + 16] = W1.T
        w1p.append(Wp)

    # life-broadcast lhsT variants: for block tb, rows rho=8tb+g (real rows
    # only), life value lives at LifeQ[q = rho-1, half = q//128].
    # Group rows into buckets by (half, 64-aligned window base). lhsT
    # R[b + (qh-b), 16g+c] = 1 -> out[16g+c, w] = life[row 8tb+g].
    life_plan = []   # per tb: list of (half, lhsT_index); lhsT K=128 base 0
    r_mats = []
    for tb in range(NB):
        plan = []
        buckets = {}
        for g in range(8):
            rho = 8 * tb + g
            if rho < 1 or rho > 256:
                continue
            q = rho - 1
            half, qh = q // 128, q % 128
            buckets.setdefault(half, []).append((g, qh))
        for half, rows in sorted(buckets.items()):
            # lhsT indexed by the pool tile's q2 layout: row (q%8)*16 + (q//8)%16
            Rm = np.zeros((128, 128), np.float32)
            for g, qh in rows:
                q2 = (qh % 8) * 16 + qh // 8
                Rm[q2, 16 * g:16 * g + 16] = 1.0
            plan.append((half, len(r_mats)))
            r_mats.append(Rm)
        life_plan.append(plan)

    import ml_dtypes
    bf16 = ml_dtypes.bfloat16
    l1_keys = sorted(l1.keys())
    l1_idx = {k: i for i, k in enumerate(l1_keys)}
    l1_stack = np.stack([l1[k] for k in l1_keys])          # [NL1, 128, 128] f32
    w1_stack = np.stack(w1p).astype(bf16)                  # [8, 128, 128] bf16
    r_stack = np.stack(r_mats).astype(bf16)                # [NR, 128, 128] bf16
    return dict(l1_stack=l1_stack, l1_idx=l1_idx, w1_stack=w1_stack,
                r_stack=r_stack, life_plan=life_plan,
                b0=b0.reshape(128, 1).astype(np.float32))


def marshal_x(img):
    """[256,256,16] image -> Xc [128, FSZ] interleaved layout."""
    xp = np.zeros((NB * 8, FW, C), np.float32)
    xp[1:257, 1:257, :] = img
    # Xc[g*16+c, t*258+w] = xp[8t+g, w, c]
    xc = xp.reshape(NB, 8, FW, C).transpose(1, 3, 0, 2).reshape(128, FSZ)
    return np.ascontiguousarray(xc)


def unmarshal_x(xc):
    """Xc [128, FSZ] -> [256,256,16] image."""
    xp = xc.reshape(8, C, NB, FW).transpose(2, 0, 3, 1)   # [NB, 8, FW, C]
    xp = xp.reshape(NB * 8, FW, C)
    return np.ascontiguousarray(xp[1:257, 1:257, :])


_PROGRAM_CACHE = {}


def kernel(x, W0, b0, W1, steps, _trace=False):
    import concourse.bass_utils as bass_utils
    steps = int(steps)
    x = np.asarray(x, dtype=np.float32)
    W0 = np.asarray(W0, dtype=np.float32)
    b0 = np.asarray(b0, dtype=np.float32)
    W1 = np.asarray(W1, dtype=np.float32)
    B = x.shape[0]
    assert x.shape == (8, H, W, C), x.shape

    wts = build_weights(W0, b0, W1)
    key = steps
    if key not in _PROGRAM_CACHE:
        from kernel_program import build_program
        _PROGRAM_CACHE[key] = build_program(steps, wts["l1_idx"],
                                            wts["life_plan"],
                                            wts["l1_stack"].shape[0],
                                            wts["r_stack"].shape[0])
    nc = _PROGRAM_CACHE[key]

    in_maps = []
    for b in range(B):
        in_maps.append({
            "xc": marshal_x(x[b]),
            "l1w": wts["l1_stack"],
            "w1w": wts["w1_stack"],
            "rw": wts["r_stack"],
            "b0w": wts["b0"],
        })
    res = bass_utils.run_bass_kernel_spmd(nc, in_maps, list(range(8)),
                                          trace=_trace)
    kernel.last_result = res
    out = np.stack([unmarshal_x(res.results[b]["out"]) for b in range(B)])
    return out.astype(np.float32)


# Allow `from kernel import *` helpers for the program builder living in a
# separate module during development; the final self-contained kernel.py
# inlines build_program (see kernel_program.py merge step).
if __name__ == "__main__":
    pass
